# revision 16
# baseline (speedup 1.0000x reference)
"""MoE routing kernel for Trainium2 (8 NeuronCores, SPMD).

Computation (see problem reference):
  h = x @ W.T + b                      [B,S,128]
  logits = h @ normalize(emb).T        [B,S,1536]
  pref_g = softmax(logits[..., g])     3 groups of 512
  dense_g = einsum('bs,bsn->bn', importance, pref_g)
  out = stack(topk_sparsify(dense) for groups [c, qk, qk, v])

Sharding: the 16384 tokens are split contiguously across 8 cores (2048
tokens/core, each core's tokens fall in a single batch b = core//2).
Each core computes its partial dense [3,512]; the host sums the two
half-sequence partials per batch and does the tiny top-k + renorm.

Default mode "v2" (rel err ~6e-5, tolerance 2e-2):
  - x and proj_w are shipped and matmul'd in fp16: halves the dominant
    HBM traffic (x is 16 MiB/core in fp32 -> 8 MiB, ~47us -> ~23us at
    358 GB/s) while keeping 10 mantissa bits, on par with the 11-bit
    f32r PE rounding. h accumulates exactly in fp32 PSUM.
  - logits matmul is a single f32r pass (the old hi/lo emb esplit cost
    an extra 10us of PE time for error far below the top-k margins).
  - all 3 groups' importance-weighted dense reductions accumulate into
    ONE shared [3,512] PSUM bank: the stationary operand for group g is
    a [128,3] column-masked w3 (col g = imp/z_g, others 0), so rows
    r != g receive +0. Frees 2 PSUM banks vs per-group accumulators.
  - exp runs per group on ScalarE with accum_out giving the softmax
    denominator for free; w3 = imp * (1/z) via DVE reciprocal+mul.
Steady-state per-iteration time (R-replica slope): ~36us vs ~62us for
the old f32r_esplit baseline. PE floor for this structure is ~34us
(mm1 13.7 + mm2 10.2 + mm3 10.2 at 2.4 GHz); ScalarE exp ~25-35us and
fp16 x DMA ~23us overlap underneath. Legacy modes (f32r_esplit | f32r |
f32) and A/B variants (v3..v5, probes) remain selectable via MOE_MODE.
"""

import os
import sys
from contextlib import ExitStack

import numpy as np

sys.path.insert(0, "/opt/trn_rl_repo")

B, S, D_MODEL, D_SPACE = 4, 4096, 2048, 128
N_GROUPS, NG, TOTAL_N = 3, 512, 1536
TOPK = (8, 4, 6)
N_CORES = 8
T_CORE = B * S // N_CORES      # 2048 tokens per core
NT = 512                       # token tile
N_TILES = T_CORE // NT         # 4
N_SUB = NT // 128              # 4 subtiles of 128 tokens
N_K = D_MODEL // 128           # 16 contraction chunks

MODE = os.environ.get("MOE_MODE", "v2")  # v2[_esplit][_x32] | f32r_esplit | f32r | f32

_cache = {}
last_results = None  # BassKernelResults of the most recent run (for test.py)


def _round11(x):
    """round fp32 to 11 explicit mantissa bits (f32r grid), RTNE."""
    u = np.ascontiguousarray(x, np.float32).view(np.uint32)
    shift = 12
    half = np.uint32(1 << (shift - 1))
    mask = np.uint32((1 << shift) - 1)
    lsb = (u >> shift) & 1
    r = (u + half - np.uint32(1) + lsb) & ~mask
    return r.view(np.float32)


def _build(mode, reps=1):
    import concourse.tile as tile
    from concourse import bacc, mybir

    f32 = mybir.dt.float32
    mm_dt = mybir.dt.float32r if mode.startswith("f32r") else f32
    esplit = mode == "f32r_esplit"
    Exp = mybir.ActivationFunctionType.Exp

    nc = bacc.Bacc("TRN2", target_bir_lowering=False, debug=False,
                   num_devices=N_CORES)

    xt = nc.declare_dram_parameter("xt", [D_MODEL, T_CORE], mm_dt, isOutput=False)
    wt = nc.declare_dram_parameter("wt", [D_MODEL, D_SPACE], mm_dt, isOutput=False)
    embt = nc.declare_dram_parameter("embt", [D_SPACE, TOTAL_N], mm_dt, isOutput=False)
    if esplit:
        embt_lo = nc.declare_dram_parameter("embt_lo", [D_SPACE, TOTAL_N], mm_dt, isOutput=False)
    bias = nc.declare_dram_parameter("bias", [D_SPACE, 1], f32, isOutput=False)
    imp = nc.declare_dram_parameter("imp", [128, T_CORE // 128], f32, isOutput=False)
    dense = nc.declare_dram_parameter("dense", [1, TOTAL_N], f32, isOutput=True)

    with tile.TileContext(nc) as tc, ExitStack() as ctx:
        const = ctx.enter_context(tc.tile_pool(name="const", bufs=1))
        xpool = ctx.enter_context(tc.tile_pool(name="x", bufs=2))
        hpool = ctx.enter_context(tc.tile_pool(name="h", bufs=2))
        epool = ctx.enter_context(tc.tile_pool(name="E", bufs=3))
        spool = ctx.enter_context(tc.tile_pool(name="small", bufs=4))
        ps_h = ctx.enter_context(tc.tile_pool(name="ph", bufs=2, space="PSUM"))
        ps_l = ctx.enter_context(tc.tile_pool(name="pl", bufs=3, space="PSUM"))
        ps_d = ctx.enter_context(tc.tile_pool(name="pd", bufs=1, space="PSUM"))

        N_CHUNK = 8                      # k-chunks per x DMA (2 MiB each)
        KC = N_K // N_CHUNK

        # weights first, in one DMA (each dma_start pays ~625ns HWDGE issue) ...
        wt_sb = const.tile([128, N_K, D_SPACE], mm_dt)
        nc.sync.dma_start(wt_sb[:],
                          wt.ap().rearrange("(k p) m -> p k m", p=128))

        # ... then tile-0's x chunks, then the remaining constants (which are
        # not needed until the first mm2/mm3, several us in).
        def load_x(t):
            chunks = []
            for kc in range(KC):
                xc = xpool.tile([128, N_CHUNK, NT], mm_dt, tag="x",
                                name=f"x_t{t}c{kc}", bufs=4)
                nc.sync.dma_start(
                    xc[:],
                    xt.ap()[kc * N_CHUNK * 128:(kc + 1) * N_CHUNK * 128,
                            t * NT:(t + 1) * NT]
                      .rearrange("(k p) n -> p k n", p=128))
                chunks.append(xc)
            return chunks

        x_chunks = load_x(0)

        embt_sb = const.tile([128, TOTAL_N], mm_dt)
        nc.sync.dma_start(embt_sb[:], embt.ap())
        if esplit:
            embt_lo_sb = const.tile([128, TOTAL_N], mm_dt)
            nc.sync.dma_start(embt_lo_sb[:], embt_lo.ap())
        bias_sb = const.tile([128, 1], f32)
        nc.sync.dma_start(bias_sb[:], bias.ap())
        imp_sb = const.tile([128, T_CORE // 128], f32)
        nc.sync.dma_start(imp_sb[:], imp.ap())

        dense_ps = [ps_d.tile([1, NG], f32, tag=f"d{g}", name=f"dense_ps{g}")
                    for g in range(N_GROUPS)]

        for rep in range(reps):
          if rep:
            x_chunks = load_x(0)
          for t in range(N_TILES):
              cur_chunks = x_chunks
              h_ps = ps_h.tile([128, NT], f32)
              for k in range(N_K):
                  nc.tensor.matmul(h_ps[:], wt_sb[:, k, :],
                                   cur_chunks[k // N_CHUNK][:, k % N_CHUNK, :],
                                   start=(k == 0), stop=(k == N_K - 1))
              if t + 1 < N_TILES:
                  x_chunks = load_x(t + 1)
              hT_sb = hpool.tile([128, NT], mm_dt)
              nc.vector.tensor_scalar_add(hT_sb[:], h_ps[:], bias_sb[:])

              for s in range(N_SUB):
                  sub = t * N_SUB + s
                  e_sb = epool.tile([128, TOTAL_N], mm_dt, tag="E")
                  z_sb = spool.tile([128, N_GROUPS], f32, tag="z")
                  for g in range(N_GROUPS):
                      lg_ps = ps_l.tile([128, NG], f32, tag="lg")
                      hslice = hT_sb[:, s * 128:(s + 1) * 128]
                      eslice = embt_sb[:, g * NG:(g + 1) * NG]
                      if esplit:
                          nc.tensor.matmul(lg_ps[:], hslice, eslice,
                                           start=True, stop=False)
                          nc.tensor.matmul(lg_ps[:], hslice,
                                           embt_lo_sb[:, g * NG:(g + 1) * NG],
                                           start=False, stop=True)
                      else:
                          nc.tensor.matmul(lg_ps[:], hslice, eslice,
                                           start=True, stop=True)
                      nc.scalar.activation(e_sb[:, g * NG:(g + 1) * NG], lg_ps[:],
                                           Exp, accum_out=z_sb[:, g:g + 1])
                  rz_sb = spool.tile([128, N_GROUPS], f32, tag="rz")
                  nc.vector.reciprocal(rz_sb[:], z_sb[:])
                  w3_sb = spool.tile([128, N_GROUPS], mm_dt, tag="w3")
                  nc.vector.tensor_scalar_mul(w3_sb[:], rz_sb[:],
                                              imp_sb[:, sub:sub + 1])
                  for g in range(N_GROUPS):
                      nc.tensor.matmul(dense_ps[g][:], w3_sb[:, g:g + 1],
                                       e_sb[:, g * NG:(g + 1) * NG],
                                       start=(sub == 0),
                                       stop=(sub == N_TILES * N_SUB - 1))

        dense_sb = spool.tile([1, TOTAL_N], f32, tag="out")
        for g in range(N_GROUPS):
            nc.vector.tensor_copy(dense_sb[0:1, g * NG:(g + 1) * NG],
                                  dense_ps[g][:])
        nc.sync.dma_start(dense.ap(), dense_sb[:])

    nc.compile()
    return nc


def _build_v2(mode, reps=1):
    """v2: fp16 x/wt for mm1 (halves x DMA), single-pass f32r logits matmul
    (esplit optional via mode), all-group dense accumulation in ONE PSUM
    bank via zero-masked w3 stationary [128,3] -> out [3,512], and
    importance folded into the exp as a per-partition bias (log imp)."""
    import concourse.tile as tile
    from concourse import bacc, mybir

    f32 = mybir.dt.float32
    f16 = mybir.dt.float16
    f32r = mybir.dt.float32r
    esplit = "esplit" in mode
    x_dt = f32r if "x32" in mode else f16
    # timing-only probes (wrong results; used to attribute the bottleneck)
    p_nomm3 = "nomm3" in mode
    p_thinexp = "thinexp" in mode
    p_nox = "nox" in mode
    # v4 = v2 + software-pipelined subtile lag + deeper lg PSUM buffering
    pipe = "pipe" in mode or mode.startswith("v4")
    v5 = mode.startswith("v5")
    # v7: batch groups {0,1} into one 1024-wide exp (accum -> z0+z1), group
    # 2 separate (accum -> z2); z1 via fp16 DVE reduce, z0 = z01 - z1.
    # Cuts ScalarE from 3x(512+352) to (1024+352)+(512+352) cycles/subtile.
    ex21 = mode.startswith("v7")
    # v9: per-group recip+w3 issued right after that group's exp, so each
    # group's dense matmul is unblocked early (v2 computes the reciprocal
    # once over all 3 z's, gating mm3 g=0 on exp g=2).
    pergroup_w3 = mode.startswith("v9")
    # v10: every matmul operand fp16 (emb, h, e, w3) — tests whether 2-byte
    # moving operands stream >1 col/cycle on the PE (mm1's x already fp16).
    f16all = mode.startswith("v10")
    e4 = mode.startswith("v11")  # v11: v2 + one extra e_sb buffer
    e_dt = f16 if (ex21 or f16all) else f32r
    mm2_dt = f16 if f16all else f32r
    pl_bufs = 5 if ("pl5" in mode or mode.startswith("v4")) else (4 if v5 else 3)
    Exp = mybir.ActivationFunctionType.Exp

    nc = bacc.Bacc("TRN2", target_bir_lowering=False, debug=False,
                   num_devices=N_CORES)

    xt = nc.declare_dram_parameter("xt", [D_MODEL, T_CORE], x_dt, isOutput=False)
    wt = nc.declare_dram_parameter("wt", [D_MODEL, D_SPACE], x_dt, isOutput=False)
    embt = nc.declare_dram_parameter("embt", [D_SPACE, TOTAL_N], mm2_dt, isOutput=False)
    if esplit:
        embt_lo = nc.declare_dram_parameter("embt_lo", [D_SPACE, TOTAL_N], mm2_dt, isOutput=False)
    bias = nc.declare_dram_parameter("bias", [D_SPACE, 1], f32, isOutput=False)
    imp = nc.declare_dram_parameter("imp", [128, T_CORE // 128], f32, isOutput=False)
    dense = nc.declare_dram_parameter("dense", [N_GROUPS, NG], f32, isOutput=True)

    with tile.TileContext(nc) as tc, ExitStack() as ctx:
        const = ctx.enter_context(tc.tile_pool(name="const", bufs=1))
        xpool = ctx.enter_context(tc.tile_pool(name="x", bufs=2))
        hpool = ctx.enter_context(tc.tile_pool(name="h", bufs=2))
        epool = ctx.enter_context(tc.tile_pool(name="E", bufs=4 if (v5 or e4) else 3))
        spool = ctx.enter_context(tc.tile_pool(name="small", bufs=6 if v5 else 4))
        ps_h = ctx.enter_context(tc.tile_pool(name="ph", bufs=1 if ex21 else 2,
                                              space="PSUM"))
        if ex21:
            ps_l2 = ctx.enter_context(tc.tile_pool(name="pl2", bufs=2, space="PSUM"))
            ps_l1 = ctx.enter_context(tc.tile_pool(name="pl1", bufs=2, space="PSUM"))
        else:
            ps_l = ctx.enter_context(tc.tile_pool(name="pl", bufs=pl_bufs, space="PSUM"))
        ps_d = ctx.enter_context(tc.tile_pool(name="pd", bufs=1, space="PSUM"))

        N_CHUNK = 8                      # k-chunks per x DMA (1 MiB each @fp16)
        KC = N_K // N_CHUNK

        wt_sb = const.tile([128, N_K, D_SPACE], x_dt)
        nc.sync.dma_start(wt_sb[:],
                          wt.ap().rearrange("(k p) m -> p k m", p=128))

        def load_x(t):
            chunks = []
            for kc in range(KC):
                xc = xpool.tile([128, N_CHUNK, NT], x_dt, tag="x",
                                name=f"x_t{t}c{kc}", bufs=4)
                nc.sync.dma_start(
                    xc[:],
                    xt.ap()[kc * N_CHUNK * 128:(kc + 1) * N_CHUNK * 128,
                            t * NT:(t + 1) * NT]
                      .rearrange("(k p) n -> p k n", p=128))
                chunks.append(xc)
            return chunks

        x_chunks = load_x(0)

        embt_sb = const.tile([128, TOTAL_N], mm2_dt)
        nc.sync.dma_start(embt_sb[:], embt.ap())
        if esplit:
            embt_lo_sb = const.tile([128, TOTAL_N], mm2_dt)
            nc.sync.dma_start(embt_lo_sb[:], embt_lo.ap())
        bias_sb = const.tile([128, 1], f32)
        nc.sync.dma_start(bias_sb[:], bias.ap())
        imp_sb = const.tile([128, T_CORE // 128], f32)
        nc.sync.dma_start(imp_sb[:], imp.ap())

        dense_ps = ps_d.tile([N_GROUPS, NG], f32, name="dense_ps")
        LAST = N_TILES * N_SUB - 1

        def do_subtiles(t, hT_sb):
              for s in range(N_SUB):
                  sub = t * N_SUB + s
                  e_sb = epool.tile([128, N_GROUPS, NG], e_dt, tag="E")
                  zq = spool.tile([128, N_GROUPS], f32, tag="z")
                  if pergroup_w3:
                      w3z_pg = spool.tile([128, N_GROUPS, N_GROUPS], e_dt, tag="w3z")
                      rz_pg = spool.tile([128, N_GROUPS], f32, tag="rz")
                      nc.vector.memset(w3z_pg[:].bitcast(
                          mybir.dt.uint16 if e_dt == f16 else mybir.dt.uint32), 0)
                  hslice = hT_sb[:, s * 128:(s + 1) * 128]
                  if ex21:
                      lg2 = ps_l2.tile([128, 2, NG], f32, tag="lg2")
                      lg1 = ps_l1.tile([128, NG], f32, tag="lg1")
                      for g in range(N_GROUPS):
                          tgt = lg1[:] if g == 2 else lg2[:, g, :]
                          nc.tensor.matmul(tgt, hslice,
                                           embt_sb[:, g * NG:(g + 1) * NG],
                                           start=True, stop=True)
                      # zq[:,0] = z0+z1 for now; zq[:,2] = z2
                      nc.scalar.activation(e_sb[:, 0:2, :], lg2[:], Exp,
                                           accum_out=zq[:, 0:1])
                      nc.scalar.activation(e_sb[:, 2, :], lg1[:], Exp,
                                           accum_out=zq[:, 2:3])
                      nc.vector.tensor_reduce(zq[:, 1:2], e_sb[:, 1, :],
                                              axis=mybir.AxisListType.X,
                                              op=mybir.AluOpType.add)
                      nc.vector.tensor_scalar(zq[:, 0:1], zq[:, 0:1],
                                              zq[:, 1:2], None,
                                              mybir.AluOpType.subtract)
                  else:
                    for g in range(N_GROUPS):
                      lg_ps = ps_l.tile([128, NG], f32, tag="lg")
                      eslice = embt_sb[:, g * NG:(g + 1) * NG]
                      if esplit:
                          nc.tensor.matmul(lg_ps[:], hslice, eslice,
                                           start=True, stop=False)
                          nc.tensor.matmul(lg_ps[:], hslice,
                                           embt_lo_sb[:, g * NG:(g + 1) * NG],
                                           start=False, stop=True)
                      else:
                          nc.tensor.matmul(lg_ps[:], hslice, eslice,
                                           start=True, stop=True)
                      if p_thinexp:
                          nc.scalar.activation(e_sb[:, g, :128],
                                               lg_ps[:, :128], Exp,
                                               accum_out=zq[:, g:g + 1])
                      else:
                          nc.scalar.activation(e_sb[:, g, :], lg_ps[:],
                                               Exp, accum_out=zq[:, g:g + 1])
                      if pergroup_w3:
                          # issue this group's recip+w3 immediately so its
                          # dense matmul is not gated on the LAST group's exp
                          nc.vector.reciprocal(rz_pg[:, g:g + 1], zq[:, g:g + 1])
                          with nc.allow_low_precision(reason="f32r bits == f32"):
                              nc.vector.tensor_scalar_mul(w3z_pg[:, g, g:g + 1],
                                                          rz_pg[:, g:g + 1],
                                                          imp_sb[:, sub:sub + 1])
                  # w3z[:, g, :] is the stationary operand for group g:
                  # column g = imp/z_g, other columns 0 so rows r != g of
                  # the shared [3, NG] accumulator receive +0.
                  if pergroup_w3:
                      w3z = w3z_pg
                  else:
                      w3z = spool.tile([128, N_GROUPS, N_GROUPS], e_dt, tag="w3z")
                      rz = spool.tile([128, N_GROUPS], f32, tag="rz")
                      nc.vector.memset(
                          w3z[:].bitcast(mybir.dt.uint16 if e_dt == f16
                                         else mybir.dt.uint32), 0)
                      nc.vector.reciprocal(rz[:], zq[:])
                      with nc.allow_low_precision(reason="f32r SBUF bits == f32"):
                          for g in range(N_GROUPS):
                              nc.vector.tensor_scalar_mul(w3z[:, g, g:g + 1],
                                                          rz[:, g:g + 1],
                                                          imp_sb[:, sub:sub + 1])
                  if not p_nomm3 or sub == 0:
                      for g in range(N_GROUPS):
                          nc.tensor.matmul(dense_ps[:], w3z[:, g, :],
                                           e_sb[:, g, :],
                                           start=(sub == 0 and g == 0),
                                           stop=(sub == (0 if p_nomm3 else LAST)
                                                 and g == N_GROUPS - 1))

        for rep in range(reps):
          if rep and not p_nox:
            x_chunks = load_x(0)
          hT_prev = None
          for t in range(N_TILES):
              cur_chunks = x_chunks
              h_ps = ps_h.tile([128, NT], f32)
              for k in range(N_K):
                  nc.tensor.matmul(h_ps[:], wt_sb[:, k, :],
                                   cur_chunks[k // N_CHUNK][:, k % N_CHUNK, :],
                                   start=(k == 0), stop=(k == N_K - 1))
              if t + 1 < N_TILES and not p_nox:
                  x_chunks = load_x(t + 1)
              hT_sb = hpool.tile([128, NT], mm2_dt)
              nc.vector.tensor_scalar_add(hT_sb[:], h_ps[:], bias_sb[:])
              if pipe:
                  if t > 0:
                      do_subtiles(t - 1, hT_prev)
                  hT_prev = hT_sb
              else:
                  do_subtiles(t, hT_sb)
          if pipe:
              do_subtiles(N_TILES - 1, hT_prev)

        dense_sb = spool.tile([N_GROUPS, TOTAL_N // N_GROUPS], f32, tag="out")
        nc.vector.tensor_copy(dense_sb[:], dense_ps[:])
        nc.sync.dma_start(dense.ap(), dense_sb[:])

    nc.compile()
    return nc


def _build_v3(mode, reps=1):
    """v3 = v2 plus:
    - software-pipelined tiles: subtile work for tile t-1 is emitted after
      mm1(t), so the PE never waits on the h bias-add; h PSUM single bank.
    - logits for all 3 groups land in ONE [128,3,512] PSUM tile (3 banks,
      double-buffered = 6 banks) -> exp is a single ScalarE instruction
      per subtile (1536 wide) instead of three 512-wide ones.
    - e stored fp16 (range-safe: max logit ~6 -> e^l <= ~400); per-group
      softmax denominators come from a 2x-mode DVE reduce instead of the
      activation accum, and w3/mm3 run in fp16.
    """
    import concourse.tile as tile
    from concourse import bacc, mybir

    f32 = mybir.dt.float32
    f16 = mybir.dt.float16
    f32r = mybir.dt.float32r
    esplit = "esplit" in mode
    p_nomm3 = "nomm3" in mode
    p_nox = "nox" in mode
    Exp = mybir.ActivationFunctionType.Exp
    AX = mybir.AxisListType.X
    ADD = mybir.AluOpType.add

    nc = bacc.Bacc("TRN2", target_bir_lowering=False, debug=False,
                   num_devices=N_CORES)

    xt = nc.declare_dram_parameter("xt", [D_MODEL, T_CORE], f16, isOutput=False)
    wt = nc.declare_dram_parameter("wt", [D_MODEL, D_SPACE], f16, isOutput=False)
    embt = nc.declare_dram_parameter("embt", [D_SPACE, TOTAL_N], f32r, isOutput=False)
    if esplit:
        embt_lo = nc.declare_dram_parameter("embt_lo", [D_SPACE, TOTAL_N], f32r, isOutput=False)
    bias = nc.declare_dram_parameter("bias", [D_SPACE, 1], f32, isOutput=False)
    imp = nc.declare_dram_parameter("imp", [128, T_CORE // 128], f32, isOutput=False)
    dense = nc.declare_dram_parameter("dense", [N_GROUPS, NG], f32, isOutput=True)

    with tile.TileContext(nc) as tc, ExitStack() as ctx:
        const = ctx.enter_context(tc.tile_pool(name="const", bufs=1))
        xpool = ctx.enter_context(tc.tile_pool(name="x", bufs=2))
        hpool = ctx.enter_context(tc.tile_pool(name="h", bufs=2))
        epool = ctx.enter_context(tc.tile_pool(name="E", bufs=3))
        spool = ctx.enter_context(tc.tile_pool(name="small", bufs=4))
        ps_h = ctx.enter_context(tc.tile_pool(name="ph", bufs=1, space="PSUM"))
        ps_l = ctx.enter_context(tc.tile_pool(name="pl", bufs=2, space="PSUM"))
        ps_d = ctx.enter_context(tc.tile_pool(name="pd", bufs=1, space="PSUM"))

        N_CHUNK = 8                      # k-chunks per x DMA (1 MiB each @fp16)
        KC = N_K // N_CHUNK

        wt_sb = const.tile([128, N_K, D_SPACE], f16)
        nc.sync.dma_start(wt_sb[:],
                          wt.ap().rearrange("(k p) m -> p k m", p=128))

        def load_x(t):
            chunks = []
            for kc in range(KC):
                xc = xpool.tile([128, N_CHUNK, NT], f16, tag="x",
                                name=f"x_t{t}c{kc}", bufs=6)
                nc.sync.dma_start(
                    xc[:],
                    xt.ap()[kc * N_CHUNK * 128:(kc + 1) * N_CHUNK * 128,
                            t * NT:(t + 1) * NT]
                      .rearrange("(k p) n -> p k n", p=128))
                chunks.append(xc)
            return chunks

        x_chunks = load_x(0)

        embt_sb = const.tile([128, TOTAL_N], f32r)
        nc.sync.dma_start(embt_sb[:], embt.ap())
        if esplit:
            embt_lo_sb = const.tile([128, TOTAL_N], f32r)
            nc.sync.dma_start(embt_lo_sb[:], embt_lo.ap())
        bias_sb = const.tile([128, 1], f32)
        nc.sync.dma_start(bias_sb[:], bias.ap())
        imp_sb = const.tile([128, T_CORE // 128], f32)
        nc.sync.dma_start(imp_sb[:], imp.ap())

        dense_ps = ps_d.tile([N_GROUPS, NG], f32, name="dense_ps")
        LAST = N_TILES * N_SUB - 1

        # w3z[:, g, :] is the stationary operand for dense group g: column
        # g = imp/z_g, other columns 0 so rows r != g of the shared [3,NG]
        # accumulator receive +0. Off-diagonal zeros are never rewritten,
        # so the memset happens once per buffer, outside the loop.
        w3z_bufs = [const.tile([128, N_GROUPS, N_GROUPS], f16, name=f"w3z{i}")
                    for i in range(4)]
        for wb in w3z_bufs:
            nc.vector.memset(wb[:], 0)

        def do_subtiles(t, hT_sb):
            for s in range(N_SUB):
                sub = t * N_SUB + s
                lg = ps_l.tile([128, N_GROUPS, NG], f32, tag="lg")
                hslice = hT_sb[:, s * 128:(s + 1) * 128]
                for g in range(N_GROUPS):
                    eslice = embt_sb[:, g * NG:(g + 1) * NG]
                    if esplit:
                        nc.tensor.matmul(lg[:, g, :], hslice, eslice,
                                         start=True, stop=False)
                        nc.tensor.matmul(lg[:, g, :], hslice,
                                         embt_lo_sb[:, g * NG:(g + 1) * NG],
                                         start=False, stop=True)
                    else:
                        nc.tensor.matmul(lg[:, g, :], hslice, eslice,
                                         start=True, stop=True)
                e_sb = epool.tile([128, N_GROUPS, NG], f16, tag="E")
                nc.scalar.activation(e_sb[:], lg[:], Exp)
                zq = spool.tile([128, N_GROUPS], f32, tag="z")
                nc.vector.tensor_reduce(zq[:], e_sb[:], axis=AX, op=ADD)
                rz = spool.tile([128, N_GROUPS], f32, tag="rz")
                nc.vector.reciprocal(rz[:], zq[:])
                w3z = w3z_bufs[sub % 4]
                for g in range(N_GROUPS):
                    nc.vector.tensor_scalar_mul(w3z[:, g, g:g + 1],
                                                rz[:, g:g + 1],
                                                imp_sb[:, sub:sub + 1])
                if not p_nomm3 or sub == 0:
                    for g in range(N_GROUPS):
                        nc.tensor.matmul(dense_ps[:], w3z[:, g, :],
                                         e_sb[:, g, :],
                                         start=(sub == 0 and g == 0),
                                         stop=(sub == (0 if p_nomm3 else LAST)
                                               and g == N_GROUPS - 1))

        for rep in range(reps):
          if rep and not p_nox:
            x_chunks = load_x(0)
          hT_prev = None
          for t in range(N_TILES):
              cur_chunks = x_chunks
              h_ps = ps_h.tile([128, NT], f32)
              for k in range(N_K):
                  nc.tensor.matmul(h_ps[:], wt_sb[:, k, :],
                                   cur_chunks[k // N_CHUNK][:, k % N_CHUNK, :],
                                   start=(k == 0), stop=(k == N_K - 1))
              if t + 1 < N_TILES and not p_nox:
                  x_chunks = load_x(t + 1)
              hT_sb = hpool.tile([128, NT], f32r)
              nc.vector.tensor_scalar_add(hT_sb[:], h_ps[:], bias_sb[:])
              if t > 0:
                  do_subtiles(t - 1, hT_prev)
              hT_prev = hT_sb
          do_subtiles(N_TILES - 1, hT_prev)

        dense_sb = spool.tile([N_GROUPS, NG], f32, tag="out")
        nc.vector.tensor_copy(dense_sb[:], dense_ps[:])
        nc.sync.dma_start(dense.ap(), dense_sb[:])

    nc.compile()
    return nc


def _build_v12(mode, reps=1):
    """v12: ScalarE/prologue-optimized variant.

    - ONE 1536-wide exp per subtile (3 groups batched), NO accum_out: the
      cost model charges a ~370ns accumulator-readout aux op per activation;
      v2's 48 accum exps put ScalarE at ~39us busy (the kernel bottleneck).
      Here ScalarE drops to ~23us and PE becomes the limiter again.
    - z via DVE tensor_reduce on fp16 e (in+out fp16, SBUF -> 4x perf mode).
    - hT/e/zq/rz/w3z all fp16 (hT fp16 also enables FWL weight loads for
      mm2; emb stays f32r: fp16 emb collapses the top-k margin to 1e-6).
    - prologue: wt/x DMAs split so mm1 k=0 starts after ~1/8 of the data,
      plus PE warm-up matmuls on zeroed SBUF during the DMA wait to start
      the HAM clock ramp early.
    Flags: z32 (f32 z reduce out, if HW fp16-accum reduce proves lossy),
    nowarm/nosplit (disable prologue tricks), tail3 (per-group exp+accum
    for the last subtile to shorten the drain).
    """
    import concourse.tile as tile
    from concourse import bacc, mybir

    f32 = mybir.dt.float32
    f16 = mybir.dt.float16
    f32r = mybir.dt.float32r
    Exp = mybir.ActivationFunctionType.Exp
    AX = mybir.AxisListType.X
    ADD = mybir.AluOpType.add
    z16 = "z16" in mode      # naive fp16-accum reduce (support-flip risk)
    warm = "nowarm" not in mode
    split = "nosplit" not in mode
    tail3 = "tail3" in mode
    # software-pipeline lag: mm3(s) is emitted after mm2/exp of s+LAG, so
    # the exp->reduce->recip->w3 chain (~1.9us) of subtile s overlaps the
    # PE slots of s+1..s+LAG instead of stalling mm3(s).
    LAG = 2
    if "l0" in mode:
        LAG = 0
    elif "l1" in mode:
        LAG = 1
    elif "l3" in mode:
        LAG = 3

    nc = bacc.Bacc("TRN2", target_bir_lowering=False, debug=False,
                   num_devices=N_CORES)

    xt = nc.declare_dram_parameter("xt", [D_MODEL, T_CORE], f16, isOutput=False)
    wt = nc.declare_dram_parameter("wt", [D_MODEL, D_SPACE], f16, isOutput=False)
    embt = nc.declare_dram_parameter("embt", [D_SPACE, TOTAL_N], f32r, isOutput=False)
    bias = nc.declare_dram_parameter("bias", [D_SPACE, 1], f32, isOutput=False)
    imp = nc.declare_dram_parameter("imp", [128, T_CORE // 128], f32, isOutput=False)
    dense = nc.declare_dram_parameter("dense", [N_GROUPS, NG], f32, isOutput=True)

    with tile.TileContext(nc) as tc, ExitStack() as ctx:
        const = ctx.enter_context(tc.tile_pool(name="const", bufs=1))
        xpool = ctx.enter_context(tc.tile_pool(name="x", bufs=2))
        hpool = ctx.enter_context(tc.tile_pool(name="h", bufs=2))
        epool = ctx.enter_context(tc.tile_pool(name="E", bufs=max(3, LAG + 2)))
        spool = ctx.enter_context(tc.tile_pool(name="small", bufs=4))
        # PSUM budget (8 banks): h 1 + logits 2x3 + dense 1
        ps_h = ctx.enter_context(tc.tile_pool(name="ph", bufs=1, space="PSUM"))
        ps_l = ctx.enter_context(tc.tile_pool(name="pl", bufs=2, space="PSUM"))
        ps_d = ctx.enter_context(tc.tile_pool(name="pd", bufs=1, space="PSUM"))

        N_CHUNK = 8                      # k-chunks per x DMA (1 MiB each @fp16)
        KC = N_K // N_CHUNK

        dense_ps = ps_d.tile([N_GROUPS, NG], f32, name="dense_ps")

        # PE warm-up: ramp the HAM clock gate on zeroed SBUF while the first
        # wt/x DMAs are in flight. Writes dense_ps with start=True each time;
        # the first real mm3 (also start=True) resets it.
        if warm:
            wst = const.tile([128, N_GROUPS], f16, name="wst")
            wmv = const.tile([128, 256], f16, name="wmv")
            nc.vector.memset(wst[:], 0)
            nc.vector.memset(wmv[:], 0)

        # weights first: split so mm1 k=0 isn't gated on the full 512KB
        wt_sb = const.tile([128, N_K, D_SPACE], f16)
        wt_r = wt.ap().rearrange("(k p) m -> p k m", p=128)
        if split:
            nc.sync.dma_start(wt_sb[:, 0:2, :], wt_r[:, 0:2, :])
            nc.sync.dma_start(wt_sb[:, 2:, :], wt_r[:, 2:, :])
        else:
            nc.sync.dma_start(wt_sb[:], wt_r)

        def load_x(t, first=False):
            chunks = []
            for kc in range(KC):
                xc = xpool.tile([128, N_CHUNK, NT], f16, tag="x",
                                name=f"x_t{t}c{kc}", bufs=4)
                src = xt.ap()[kc * N_CHUNK * 128:(kc + 1) * N_CHUNK * 128,
                              t * NT:(t + 1) * NT] \
                    .rearrange("(k p) n -> p k n", p=128)
                if first and kc == 0:
                    nc.sync.dma_start(xc[:, 0:2, :], src[:, 0:2, :])
                    nc.sync.dma_start(xc[:, 2:, :], src[:, 2:, :])
                else:
                    nc.sync.dma_start(xc[:], src)
                chunks.append(xc)
            return chunks

        x_chunks = load_x(0, first=split)

        embt_sb = const.tile([128, TOTAL_N], f32r)
        nc.sync.dma_start(embt_sb[:], embt.ap())
        bias_sb = const.tile([128, 1], f32)
        nc.sync.dma_start(bias_sb[:], bias.ap())
        imp_sb = const.tile([128, T_CORE // 128], f32)
        nc.sync.dma_start(imp_sb[:], imp.ap())

        if warm:
            for _ in range(8):
                nc.tensor.matmul(dense_ps[:, :256], wst[:], wmv[:],
                                 start=True, stop=True)

        LAST = N_TILES * N_SUB - 1

        # w3z[:, g, :]: stationary for dense group g; col g = imp/z_g, other
        # cols 0 (+0 into rows r != g of the shared [3,NG] accumulator).
        # Off-diagonal zeros never rewritten -> memset once per buffer.
        w3z_bufs = [const.tile([128, N_GROUPS, N_GROUPS], f16, name=f"w3z{i}")
                    for i in range(4)]
        for wb in w3z_bufs:
            nc.vector.memset(wb[:], 0)

        def stage_a(sub, hT_sb):
            """mm2 + exp + z-chain for one subtile; returns (sub, e, w3z)."""
            s = sub % N_SUB
            lg = ps_l.tile([128, N_GROUPS, NG], f32, tag="lg")
            hslice = hT_sb[:, s * 128:(s + 1) * 128]
            for g in range(N_GROUPS):
                nc.tensor.matmul(lg[:, g, :], hslice,
                                 embt_sb[:, g * NG:(g + 1) * NG],
                                 start=True, stop=True)
            e_sb = epool.tile([128, N_GROUPS, NG], f16, tag="E")
            zq = spool.tile([128, N_GROUPS], f16 if z16 else f32, tag="z")
            if tail3 and sub == LAST:
                for g in range(N_GROUPS):
                    nc.scalar.activation(e_sb[:, g, :], lg[:, g, :],
                                         Exp, accum_out=zq[:, g:g + 1])
            else:
                nc.scalar.activation(e_sb[:], lg[:], Exp)
                with nc.allow_low_precision(reason="z16 is an A/B probe"):
                    nc.vector.tensor_reduce(zq[:], e_sb[:], axis=AX, op=ADD)
            rz = spool.tile([128, N_GROUPS], f16, tag="rz")
            with nc.allow_low_precision(reason="1/z fp16: 5e-4 rms, "
                                        "attenuated 40x by token avg"):
                nc.vector.reciprocal(rz[:], zq[:])
            w3z = w3z_bufs[sub % 4]
            with nc.allow_low_precision(reason="w3 fp16 ok (see above)"):
                for g in range(N_GROUPS):
                    nc.vector.tensor_scalar_mul(w3z[:, g, g:g + 1],
                                                rz[:, g:g + 1],
                                                imp_sb[:, sub:sub + 1])
            return (sub, e_sb, w3z)

        def stage_b(item):
            sub, e_sb, w3z = item
            for g in range(N_GROUPS):
                nc.tensor.matmul(dense_ps[:], w3z[:, g, :],
                                 e_sb[:, g, :],
                                 start=(sub == 0 and g == 0),
                                 stop=(sub == LAST and g == N_GROUPS - 1))

        from collections import deque
        pending = deque()
        for rep in range(reps):
          if rep:
            x_chunks = load_x(0)
          for t in range(N_TILES):
              cur_chunks = x_chunks
              h_ps = ps_h.tile([128, NT], f32)
              for k in range(N_K):
                  nc.tensor.matmul(h_ps[:], wt_sb[:, k, :],
                                   cur_chunks[k // N_CHUNK][:, k % N_CHUNK, :],
                                   start=(k == 0), stop=(k == N_K - 1))
              if t + 1 < N_TILES:
                  x_chunks = load_x(t + 1)
              hT_sb = hpool.tile([128, NT], f32r)
              nc.vector.tensor_scalar_add(hT_sb[:], h_ps[:], bias_sb[:])
              for s in range(N_SUB):
                  pending.append(stage_a(t * N_SUB + s, hT_sb))
                  if len(pending) > LAG:
                      stage_b(pending.popleft())
          while pending:
              stage_b(pending.popleft())

        dense_sb = spool.tile([N_GROUPS, NG], f32, tag="out")
        nc.vector.tensor_copy(dense_sb[:], dense_ps[:])
        nc.sync.dma_start(dense.ap(), dense_sb[:])

    nc.compile()
    return nc


def _build_v13(mode, reps=1):
    """v13 = v12 pipeline + mm1 spread across subtile slots.

    v12's remaining stalls: (a) mm1 runs as a 3.4us burst per tile while
    ScalarE/DVE idle, then the 4-subtile phase paces at the DVE z-chain
    (~1.8us/subtile) while the PE idles; (b) prologue serializes on coarse
    DMAs. Here each subtile slot carries mm2(s) + mm3(s-LAG) + a 5-6 matmul
    chunk of the NEXT tile's mm1, so the PE slot (~2.1us) exceeds ScalarE
    (~1.6us) and DVE (~1.8us) per-subtile loads: the PE becomes the
    pace-setter at its roofline. Tile 1's mm1 stays a burst (x DMA
    bandwidth can't feed a tile-early prefetch that soon); tiles 2+ spread.
    Prologue DMAs are split fine-grained (wt k0-1, x0 k0-1, emb per-group,
    x1 in 4 pieces) so mm1 starts after ~0.3 MiB instead of ~2.5 MiB.
    """
    import concourse.tile as tile
    from concourse import bacc, mybir
    from collections import deque

    f32 = mybir.dt.float32
    f16 = mybir.dt.float16
    f32r = mybir.dt.float32r
    Exp = mybir.ActivationFunctionType.Exp
    AX = mybir.AxisListType.X
    ADD = mybir.AluOpType.add
    warm = "nowarm" not in mode
    tail3 = "tail3" in mode
    LAG = 2
    if "l1" in mode:
        LAG = 1
    elif "l3" in mode:
        LAG = 3

    nc = bacc.Bacc("TRN2", target_bir_lowering=False, debug=False,
                   num_devices=N_CORES)

    xt = nc.declare_dram_parameter("xt", [D_MODEL, T_CORE], f16, isOutput=False)
    wt = nc.declare_dram_parameter("wt", [D_MODEL, D_SPACE], f16, isOutput=False)
    embt = nc.declare_dram_parameter("embt", [D_SPACE, TOTAL_N], f32r, isOutput=False)
    bias = nc.declare_dram_parameter("bias", [D_SPACE, 1], f32, isOutput=False)
    imp = nc.declare_dram_parameter("imp", [128, T_CORE // 128], f32, isOutput=False)
    dense = nc.declare_dram_parameter("dense", [N_GROUPS, NG], f32, isOutput=True)

    with tile.TileContext(nc) as tc, ExitStack() as ctx:
        const = ctx.enter_context(tc.tile_pool(name="const", bufs=1))
        xpool = ctx.enter_context(tc.tile_pool(name="x", bufs=2))
        hpool = ctx.enter_context(tc.tile_pool(name="h", bufs=2))
        epool = ctx.enter_context(tc.tile_pool(name="E", bufs=max(3, LAG + 2)))
        spool = ctx.enter_context(tc.tile_pool(name="small", bufs=4))
        # PSUM budget (8 banks): h 1 + logits 2x3 + dense 1
        ps_h = ctx.enter_context(tc.tile_pool(name="ph", bufs=1, space="PSUM"))
        ps_l = ctx.enter_context(tc.tile_pool(name="pl", bufs=2, space="PSUM"))
        ps_d = ctx.enter_context(tc.tile_pool(name="pd", bufs=1, space="PSUM"))

        dense_ps = ps_d.tile([N_GROUPS, NG], f32, name="dense_ps")

        if warm:
            wst = const.tile([128, N_GROUPS], f16, name="wst")
            wmv = const.tile([128, 256], f16, name="wmv")
            nc.vector.memset(wst[:], 0)
            nc.vector.memset(wmv[:], 0)

        # --- fine-grained prologue DMAs, priority order ---
        wt_r = wt.ap().rearrange("(k p) m -> p k m", p=128)
        wt_a = const.tile([128, 2, D_SPACE], f16, name="wt_a")
        nc.sync.dma_start(wt_a[:], wt_r[:, 0:2, :])

        def load_x(t, pieces):
            """DMA x tile t in the given ktile-count pieces; returns a list
            mapping global k -> (sbuf_tile, local_idx)."""
            chunks = []
            k0 = 0
            for i, nk in enumerate(pieces):
                xc = xpool.tile([128, nk, NT], f16, tag=f"xp{nk}",
                                name=f"x_t{t}p{i}", bufs=max(2, 32 // nk))
                nc.sync.dma_start(
                    xc[:],
                    xt.ap()[k0 * 128:(k0 + nk) * 128, t * NT:(t + 1) * NT]
                      .rearrange("(k p) n -> p k n", p=128))
                chunks.extend((xc, j) for j in range(nk))
                k0 += nk
            return chunks

        x_cur = load_x(0, (2, 6, 8))
        emb_g = []
        for g in range(N_GROUPS):
            eg = const.tile([128, NG], f32r, name=f"emb_g{g}")
            if g == 0:
                nc.sync.dma_start(eg[:], embt.ap()[:, 0:NG])
            emb_g.append(eg)
        wt_b = const.tile([128, N_K - 2, D_SPACE], f16, name="wt_b")
        nc.sync.dma_start(wt_b[:], wt_r[:, 2:, :])
        for g in range(1, N_GROUPS):
            nc.sync.dma_start(emb_g[g][:], embt.ap()[:, g * NG:(g + 1) * NG])
        bias_sb = const.tile([128, 1], f32)
        nc.sync.dma_start(bias_sb[:], bias.ap())
        imp_sb = const.tile([128, T_CORE // 128], f32)
        nc.sync.dma_start(imp_sb[:], imp.ap())
        x_next = load_x(1, (4, 4, 4, 4))

        def wtk(k):
            return wt_a[:, k, :] if k < 2 else wt_b[:, k - 2, :]

        if warm:
            for _ in range(8):
                nc.tensor.matmul(dense_ps[:, :256], wst[:], wmv[:],
                                 start=True, stop=True)

        LAST = N_TILES * N_SUB - 1
        w3z_bufs = [const.tile([128, N_GROUPS, N_GROUPS], f16, name=f"w3z{i}")
                    for i in range(max(4, LAG + 2))]
        for wb in w3z_bufs:
            nc.vector.memset(wb[:], 0)

        def mm1(h_ps, chunks, k0, k1):
            for k in range(k0, k1):
                xc, j = chunks[k]
                nc.tensor.matmul(h_ps[:], wtk(k), xc[:, j, :],
                                 start=(k == 0), stop=(k == N_K - 1))

        def bias_add(h_ps):
            hT = hpool.tile([128, NT], f32r, tag="hT")
            nc.vector.tensor_scalar_add(hT[:], h_ps[:], bias_sb[:])
            return hT

        def stage_a(sub, hT_sb):
            s = sub % N_SUB
            lg = ps_l.tile([128, N_GROUPS, NG], f32, tag="lg")
            hslice = hT_sb[:, s * 128:(s + 1) * 128]
            for g in range(N_GROUPS):
                nc.tensor.matmul(lg[:, g, :], hslice, emb_g[g][:],
                                 start=True, stop=True)
            e_sb = epool.tile([128, N_GROUPS, NG], f16, tag="E")
            zq = spool.tile([128, N_GROUPS], f32, tag="z")
            if tail3 and sub == LAST:
                for g in range(N_GROUPS):
                    nc.scalar.activation(e_sb[:, g, :], lg[:, g, :],
                                         Exp, accum_out=zq[:, g:g + 1])
            else:
                nc.scalar.activation(e_sb[:], lg[:], Exp)
                nc.vector.tensor_reduce(zq[:], e_sb[:], axis=AX, op=ADD)
            rz = spool.tile([128, N_GROUPS], f16, tag="rz")
            with nc.allow_low_precision(reason="1/z fp16: 5e-4 rms, "
                                        "attenuated 40x by token avg"):
                nc.vector.reciprocal(rz[:], zq[:])
            w3z = w3z_bufs[sub % len(w3z_bufs)]
            with nc.allow_low_precision(reason="w3 fp16 ok (see above)"):
                for g in range(N_GROUPS):
                    nc.vector.tensor_scalar_mul(w3z[:, g, g:g + 1],
                                                rz[:, g:g + 1],
                                                imp_sb[:, sub:sub + 1])
            return (sub, e_sb, w3z)

        def stage_b(item):
            sub, e_sb, w3z = item
            for g in range(N_GROUPS):
                nc.tensor.matmul(dense_ps[:], w3z[:, g, :], e_sb[:, g, :],
                                 start=(sub == 0 and g == 0),
                                 stop=(sub == LAST and g == N_GROUPS - 1))

        # per-slot k-ranges for the spread mm1 of the NEXT tile
        SPREAD = ((0, 6), (6, 11), (11, 16), (16, 16))
        pending = deque()
        for rep in range(reps):
            if rep:
                x_cur = load_x(0, (8, 8))
                x_next = load_x(1, (8, 8))
            # tile 0 mm1: DMA-paced burst
            h_cur = ps_h.tile([128, NT], f32, tag="h", name="h0")
            mm1(h_cur, x_cur, 0, N_K)
            hT_cur = bias_add(h_cur)
            h_next = None
            hT_next = None
            for t in range(N_TILES):
                spread = t >= 1          # tile t carries mm1 of tile t+1
                burst_next = (t == 0)    # mm1(1) bursts at tile-1 start
                for s in range(N_SUB):
                    sub = t * N_SUB + s
                    if s == 0 and t + 2 < N_TILES:
                        x_next2 = load_x(t + 2, (8, 8))
                    if spread and t + 1 < N_TILES:
                        if s == 0:
                            h_next = ps_h.tile([128, NT], f32, tag="h",
                                               name=f"h{t + 1}")
                        k0, k1 = SPREAD[s]
                        mm1(h_next, x_next, k0, k1)
                    pending.append(stage_a(sub, hT_cur))
                    if len(pending) > LAG:
                        stage_b(pending.popleft())
                    if spread and t + 1 < N_TILES and s == 2:
                        hT_next = bias_add(h_next)
                # tile boundary
                if burst_next:
                    h_next = ps_h.tile([128, NT], f32, tag="h", name="h1")
                    mm1(h_next, x_next, 0, N_K)
                    hT_next = bias_add(h_next)
                if t + 1 < N_TILES:
                    hT_cur = hT_next
                    x_next = x_next2 if t + 2 < N_TILES else None
            while pending:
                stage_b(pending.popleft())

        dense_sb = spool.tile([N_GROUPS, NG], f32, tag="out")
        nc.vector.tensor_copy(dense_sb[:], dense_ps[:])
        nc.sync.dma_start(dense.ap(), dense_sb[:])

    nc.compile()
    return nc


def _build_v14(mode, reps=1):
    """v14 = v12 burst structure + dual-ring DMAs + early x prefetch.

    Constants (wt/emb/bias/imp) issue on the Activation HWDGE ring so the
    x stream (SP ring) is never queued behind them; x0/x1 are issued in the
    prologue, x(t+2) at tile-t start. The tile scheduler then overlaps
    mm1(t+1) into tile t's subtile phase on its own wherever x has landed.
    stage_b (dense mm3) lags LAG subtiles behind stage_a (mm2+exp+z chain).
    """
    import concourse.tile as tile
    from concourse import bacc, mybir
    from collections import deque

    f32 = mybir.dt.float32
    f16 = mybir.dt.float16
    f32r = mybir.dt.float32r
    Exp = mybir.ActivationFunctionType.Exp
    AX = mybir.AxisListType.X
    ADD = mybir.AluOpType.add
    warm = "nowarm" not in mode
    tail3 = "tail3" in mode
    LAG = 2
    if "l1" in mode:
        LAG = 1
    elif "l3" in mode:
        LAG = 3
    elif "l4" in mode:
        LAG = 4

    nc = bacc.Bacc("TRN2", target_bir_lowering=False, debug=False,
                   num_devices=N_CORES)

    xt = nc.declare_dram_parameter("xt", [D_MODEL, T_CORE], f16, isOutput=False)
    wt = nc.declare_dram_parameter("wt", [D_MODEL, D_SPACE], f16, isOutput=False)
    embt = nc.declare_dram_parameter("embt", [D_SPACE, TOTAL_N], f32r, isOutput=False)
    bias = nc.declare_dram_parameter("bias", [D_SPACE, 1], f32, isOutput=False)
    imp = nc.declare_dram_parameter("imp", [128, T_CORE // 128], f32, isOutput=False)
    dense = nc.declare_dram_parameter("dense", [N_GROUPS, NG], f32, isOutput=True)

    with tile.TileContext(nc) as tc, ExitStack() as ctx:
        const = ctx.enter_context(tc.tile_pool(name="const", bufs=1))
        xpool = ctx.enter_context(tc.tile_pool(name="x", bufs=2))
        hpool = ctx.enter_context(tc.tile_pool(name="h", bufs=2))
        epool = ctx.enter_context(tc.tile_pool(name="E", bufs=max(3, LAG + 2)))
        spool = ctx.enter_context(tc.tile_pool(name="small", bufs=4))
        ps_h = ctx.enter_context(tc.tile_pool(name="ph", bufs=1, space="PSUM"))
        ps_l = ctx.enter_context(tc.tile_pool(name="pl", bufs=2, space="PSUM"))
        ps_d = ctx.enter_context(tc.tile_pool(name="pd", bufs=1, space="PSUM"))

        dense_ps = ps_d.tile([N_GROUPS, NG], f32, name="dense_ps")

        if warm:
            wst = const.tile([128, N_GROUPS], f16, name="wst")
            wmv = const.tile([128, 256], f16, name="wmv")
            nc.vector.memset(wst[:], 0)
            nc.vector.memset(wmv[:], 0)

        # constants on the Activation HWDGE ring
        wt_r = wt.ap().rearrange("(k p) m -> p k m", p=128)
        wt_a = const.tile([128, 2, D_SPACE], f16, name="wt_a")
        nc.scalar.dma_start(wt_a[:], wt_r[:, 0:2, :])
        emb_g = []
        for g in range(N_GROUPS):
            eg = const.tile([128, NG], f32r, name=f"emb_g{g}")
            emb_g.append(eg)
        nc.scalar.dma_start(emb_g[0][:], embt.ap()[:, 0:NG])
        wt_b = const.tile([128, N_K - 2, D_SPACE], f16, name="wt_b")
        nc.scalar.dma_start(wt_b[:], wt_r[:, 2:, :])
        for g in range(1, N_GROUPS):
            nc.scalar.dma_start(emb_g[g][:], embt.ap()[:, g * NG:(g + 1) * NG])
        bias_sb = const.tile([128, 1], f32)
        nc.scalar.dma_start(bias_sb[:], bias.ap())
        imp_sb = const.tile([128, T_CORE // 128], f32)
        nc.scalar.dma_start(imp_sb[:], imp.ap())

        # x stream on the SP ring
        def load_x(t, pieces):
            chunks = []
            k0 = 0
            for i, nk in enumerate(pieces):
                xc = xpool.tile([128, nk, NT], f16, tag=f"xp{nk}",
                                name=f"x_t{t}p{i}", bufs=max(2, 32 // nk))
                nc.sync.dma_start(
                    xc[:],
                    xt.ap()[k0 * 128:(k0 + nk) * 128, t * NT:(t + 1) * NT]
                      .rearrange("(k p) n -> p k n", p=128))
                chunks.extend((xc, j) for j in range(nk))
                k0 += nk
            return chunks

        x_cur = load_x(0, (2, 6, 8))
        x_next = load_x(1, (4, 4, 4, 4))

        def wtk(k):
            return wt_a[:, k, :] if k < 2 else wt_b[:, k - 2, :]

        if warm:
            for _ in range(8):
                nc.tensor.matmul(dense_ps[:, :256], wst[:], wmv[:],
                                 start=True, stop=True)

        LAST = N_TILES * N_SUB - 1
        w3z_bufs = [const.tile([128, N_GROUPS, N_GROUPS], f16, name=f"w3z{i}")
                    for i in range(max(4, LAG + 2))]
        for wb in w3z_bufs:
            nc.vector.memset(wb[:], 0)

        def bias_add(h_ps):
            hT = hpool.tile([128, NT], f32r, tag="hT")
            nc.vector.tensor_scalar_add(hT[:], h_ps[:], bias_sb[:])
            return hT

        def stage_a(sub, hT_sb):
            s = sub % N_SUB
            lg = ps_l.tile([128, N_GROUPS, NG], f32, tag="lg")
            hslice = hT_sb[:, s * 128:(s + 1) * 128]
            for g in range(N_GROUPS):
                nc.tensor.matmul(lg[:, g, :], hslice, emb_g[g][:],
                                 start=True, stop=True)
            e_sb = epool.tile([128, N_GROUPS, NG], f16, tag="E")
            zq = spool.tile([128, N_GROUPS], f32, tag="z")
            if tail3 and sub == LAST:
                for g in range(N_GROUPS):
                    nc.scalar.activation(e_sb[:, g, :], lg[:, g, :],
                                         Exp, accum_out=zq[:, g:g + 1])
            else:
                nc.scalar.activation(e_sb[:], lg[:], Exp)
                nc.vector.tensor_reduce(zq[:], e_sb[:], axis=AX, op=ADD)
            rz = spool.tile([128, N_GROUPS], f16, tag="rz")
            with nc.allow_low_precision(reason="1/z fp16: 5e-4 rms, "
                                        "attenuated 40x by token avg"):
                nc.vector.reciprocal(rz[:], zq[:])
            w3z = w3z_bufs[sub % len(w3z_bufs)]
            with nc.allow_low_precision(reason="w3 fp16 ok (see above)"):
                for g in range(N_GROUPS):
                    nc.vector.tensor_scalar_mul(w3z[:, g, g:g + 1],
                                                rz[:, g:g + 1],
                                                imp_sb[:, sub:sub + 1])
            return (sub, e_sb, w3z)

        def stage_b(item):
            sub, e_sb, w3z = item
            for g in range(N_GROUPS):
                nc.tensor.matmul(dense_ps[:], w3z[:, g, :], e_sb[:, g, :],
                                 start=(sub == 0 and g == 0),
                                 stop=(sub == LAST and g == N_GROUPS - 1))

        pending = deque()
        for rep in range(reps):
            if rep:
                x_cur = load_x(0, (8, 8))
                x_next = load_x(1, (8, 8))
            for t in range(N_TILES):
                h_ps = ps_h.tile([128, NT], f32, tag="h", name=f"h{t}")
                for k in range(N_K):
                    xc, j = x_cur[k]
                    nc.tensor.matmul(h_ps[:], wtk(k), xc[:, j, :],
                                     start=(k == 0), stop=(k == N_K - 1))
                if t + 2 < N_TILES:
                    x_next2 = load_x(t + 2, (8, 8))
                hT = bias_add(h_ps)
                for s in range(N_SUB):
                    pending.append(stage_a(t * N_SUB + s, hT))
                    if len(pending) > LAG:
                        stage_b(pending.popleft())
                if t + 1 < N_TILES:
                    x_cur = x_next
                    x_next = x_next2 if t + 2 < N_TILES else None
            while pending:
                stage_b(pending.popleft())

        dense_sb = spool.tile([N_GROUPS, NG], f32, tag="out")
        nc.vector.tensor_copy(dense_sb[:], dense_ps[:])
        nc.sync.dma_start(dense.ap(), dense_sb[:])

    nc.compile()
    return nc


def _get_nc(mode):
    if mode not in _cache:
        _cache[mode] = build(mode)
    return _cache[mode]


def _topk_sparsify(w, k):
    # match jax.lax.top_k tie-breaking (lower index wins) via stable argsort
    idx = np.argsort(-w, kind="stable")[:k]
    sp = np.zeros_like(w)
    sp[idx] = w[idx]
    return sp / (sp.sum(dtype=np.float32) + np.float32(1e-8))


def build(mode, reps=1):
    if mode.startswith("v14"):
        return _build_v14(mode, reps)
    if mode.startswith("v13"):
        return _build_v13(mode, reps)
    if mode.startswith("v12"):
        return _build_v12(mode, reps)
    if mode.startswith("v3"):
        return _build_v3(mode, reps)
    if mode.startswith("v"):
        return _build_v2(mode, reps)
    return _build(mode, reps)


def prepare_in_maps(inputs, mode=None):
    mode = mode or MODE
    x = np.asarray(inputs["x"], np.float32)
    importance = np.asarray(inputs["importance"], np.float32)
    proj_w = np.asarray(inputs["proj_w"], np.float32)
    proj_b = np.asarray(inputs["proj_b"], np.float32)
    neuron_emb = np.asarray(inputs["neuron_emb"], np.float32)

    if mode.startswith("v"):
        nrm = np.sqrt((neuron_emb ** 2).sum(axis=-1, keepdims=True, dtype=np.float32))
        embn = neuron_emb / np.maximum(nrm, np.float32(1e-12))
        embT = np.ascontiguousarray(embn.T, np.float32)        # [128, 1536]
        x_np_dt = np.float32 if "x32" in mode else np.float16
        emb_np_dt = np.float16 if mode.startswith("v10") else np.float32
        wt_host = np.ascontiguousarray(proj_w.T).astype(x_np_dt)
        bias_host = np.ascontiguousarray(proj_b.reshape(D_SPACE, 1), np.float32)
        x_flat = x.reshape(B * S, D_MODEL)
        imp_flat = importance.reshape(B * S)
        esplit = "esplit" in mode
        if esplit:
            embT_hi = _round11(embT)
            embT_lo = _round11(embT - embT_hi)
        in_maps = []
        for c in range(N_CORES):
            sl = slice(c * T_CORE, (c + 1) * T_CORE)
            m = {
                "xt": np.ascontiguousarray(x_flat[sl].T).astype(x_np_dt),
                "wt": wt_host,
                "embt": (embT_hi if esplit else embT).astype(emb_np_dt),
                "bias": bias_host,
                "imp": np.ascontiguousarray(
                    imp_flat[sl].reshape(T_CORE // 128, 128).T),
            }
            if esplit:
                m["embt_lo"] = embT_lo
            in_maps.append(m)
        return in_maps

    rnd = _round11 if mode.startswith("f32r") else (lambda a: np.ascontiguousarray(a, np.float32))

    nrm = np.sqrt((neuron_emb ** 2).sum(axis=-1, keepdims=True, dtype=np.float32))
    embn = neuron_emb / np.maximum(nrm, np.float32(1e-12))
    embT = np.ascontiguousarray(embn.T)                       # [128, 1536]
    embT_hi = rnd(embT)
    wt_host = rnd(proj_w.T)                                   # [2048, 128]
    bias_host = np.ascontiguousarray(proj_b.reshape(D_SPACE, 1), np.float32)

    x_flat = x.reshape(B * S, D_MODEL)
    imp_flat = importance.reshape(B * S)

    in_maps = []
    for c in range(N_CORES):
        sl = slice(c * T_CORE, (c + 1) * T_CORE)
        m = {
            "xt": rnd(x_flat[sl].T),                          # [2048, 2048]
            "wt": wt_host,
            "embt": embT_hi,
            "bias": bias_host,
            "imp": np.ascontiguousarray(
                imp_flat[sl].reshape(T_CORE // 128, 128).T),  # [128, 16]
        }
        if mode == "f32r_esplit":
            m["embt_lo"] = _round11(embT - embT_hi)
        in_maps.append(m)
    return in_maps


def kernel(**inputs):
    from concourse.bass_utils import run_bass_kernel_spmd
    global last_results

    mode = MODE
    nc = _get_nc(mode)
    in_maps = prepare_in_maps(inputs, mode)

    trace = bool(int(os.environ.get("MOE_TRACE", "0")))
    res = run_bass_kernel_spmd(nc, in_maps, core_ids=list(range(N_CORES)),
                               trace=trace)
    last_results = res

    parts = np.stack([res.results[c]["dense"].reshape(N_GROUPS, NG)
                      for c in range(N_CORES)])                # [8,3,512]
    dense = (parts[0::2] + parts[1::2]).transpose(1, 0, 2)     # [3,B,512]

    cw = np.stack([_topk_sparsify(dense[0, b], TOPK[0]) for b in range(B)])
    qw = np.stack([_topk_sparsify(dense[1, b], TOPK[1]) for b in range(B)])
    vw = np.stack([_topk_sparsify(dense[2, b], TOPK[2]) for b in range(B)])
    return np.stack([cw, qw, qw, vw]).astype(np.float32)

